# revision 4
# baseline (speedup 1.0000x reference)
"""Trainium2 Bass kernel for nn_Attention_12558484374097.

Single-head attention block (ViT-style, B=32, N=577, C=768):
    qkv = x @ w_qkv.T                         [B,N,3C]
    q,k,v split; attn = softmax(q @ k.T / sqrt(C))
    y = attn @ v
    y = y.transpose(0,2,1).reshape(B,N,C)     (data permutation, faithful bug)
    out = y @ w_proj.T + b_proj
    token_attn = attn[:, 0, 1:].reshape(B,1,24,24)

Sharding: data-parallel over batch, 4 batches per core on 8 cores; weights
replicated. Matmuls run as float32r (raw fp32 bits, TF32-class precision,
full PE rate with even moving chunks >=256).

Device-side design:
  - all matmuls contract over the partition axis, so x ships host-transposed
    as xT[c, n] and weights ship host-transposed (w_qkvT, w_projT).
  - a matmul's PSUM output must stay inside one 2 KiB bank, and fp32r needs
    even moving sizes >=256 for full rate: every output row is split into two
    equal chunks, each landing in its own bank of a [128, 2, 512] PSUM tile;
    SBUF tensors stay contiguous (580/768 wide) and evacuate with one 3D copy.
  - softmax runs in transposed orientation sT[m, n] = k @ q.T so exp needs no
    transpose; the denominator r[n] = sum_m exp(sT) comes from a ones-vector
    matmul, 1/r is broadcast across partitions with a rank-1 matmul, and the
    normalization is folded into the yT PSUM evacuation (y's free axis is n).
  - the reshape(transpose(y)) permutation: store yT[c, n] c-major to DRAM,
    reload the same bytes as z[n', c'], PE-transpose tiles into zT[c', n']
    for the projection matmul.
"""

import numpy as np

import concourse.bass as bass
import concourse.mybir as mybir
import concourse.tile as tile
from concourse import bacc
from concourse.bass_utils import run_bass_kernel_spmd
from concourse.masks import make_identity

B = 32
N = 577  # tokens
C = 768  # dim
NCORES = 8
BPC = B // NCORES  # batches per core
NP = 580  # padded token count: even halves of 290, each within a PSUM bank
NH = NP // 2  # 290
CH = C // 2  # 384
KT = C // 128  # 6 contraction tiles over C
F32 = mybir.dt.float32
F32R = mybir.dt.float32r
SCALE = C ** -0.5

# token partition chunks
MC = [(0, 128), (128, 128), (256, 128), (384, 128), (512, 65)]


def build_kernel():
    nc = bacc.Bacc(None, target_bir_lowering=False)

    xT_d = nc.dram_tensor("xT", [BPC, C, NP], F32R, kind="ExternalInput")
    wq_d = nc.dram_tensor("w_qkvT", [C, 3 * C], F32R, kind="ExternalInput")
    wp_d = nc.dram_tensor("w_projT", [C, C], F32R, kind="ExternalInput")
    bp_d = nc.dram_tensor("b_proj", [1, C], F32R, kind="ExternalInput")
    ones_d = nc.dram_tensor("ones", [128, 128], F32R, kind="ExternalInput")
    out_d = nc.dram_tensor("out", [BPC, N, C], F32, kind="ExternalOutput")
    ta_d = nc.dram_tensor("token_attn", [BPC, N - 1], F32, kind="ExternalOutput")

    def two(ap3, w):
        """The two [.., w] bank views of a [P, 2, 512] psum tile."""
        return (ap3[:, 0, :w], ap3[:, 1, :w])

    with tile.TileContext(nc) as tc:
        with (
            tc.tile_pool(name="consts", bufs=1) as consts,
            tc.tile_pool(name="work", bufs=1) as work,
            tc.tile_pool(name="psum", bufs=1, space="PSUM") as psum,
            tc.tile_pool(name="dram", bufs=1, space="DRAM") as dram,
        ):
            # ---- constants ----
            wq_sb = consts.tile([128, KT, 3 * C], F32R, name="wq_sb")
            nc.sync.dma_start(out=wq_sb, in_=wq_d.ap().rearrange("(k p) o -> p k o", p=128))
            wp_sb = consts.tile([128, KT, C], F32R, name="wp_sb")
            nc.sync.dma_start(out=wp_sb, in_=wp_d.ap().rearrange("(k p) o -> p k o", p=128))
            bp_sb = consts.tile([1, C], F32R, name="bp_sb")
            nc.sync.dma_start(out=bp_sb, in_=bp_d.ap())
            ones_sb = consts.tile([128, 128], F32R, name="ones_sb")
            nc.sync.dma_start(out=ones_sb, in_=ones_d.ap())
            ident = consts.tile([128, 128], F32, name="ident")
            make_identity(nc, ident)

            for b in range(BPC):
                # ---- load xT[c, n] ----
                xT = work.tile([128, KT, NP], F32R, name=f"xT{b}", tag="xT", bufs=1)
                nc.sync.dma_start(
                    out=xT, in_=xT_d.ap()[b].rearrange("(k p) n -> p k n", p=128)
                )

                # ---- qkT[o, n] = w_qkvT[:, :2C].T @ xT   (12 o-tiles) ----
                qk = work.tile([128, 12, NP], F32R, name=f"qk{b}", tag="qk", bufs=1)
                for o in range(12):
                    ps = psum.tile([128, 2, 512], F32, name=f"psqk{b}_{o}", tag="mm", bufs=2)
                    for k in range(KT):
                        for h, dst in enumerate(two(ps, NH)):
                            nc.tensor.matmul(
                                dst,
                                wq_sb[:, k, o * 128 : (o + 1) * 128],
                                xT[:, k, h * NH : (h + 1) * NH],
                                start=(k == 0),
                                stop=(k == KT - 1),
                            )
                    nc.vector.tensor_copy(
                        qk[:, o, :].rearrange("p (h w) -> p h w", h=2), ps[:, :, :NH]
                    )

                # ---- v[m, c] = xT.T @ w_vT   (5 m-chunks) ----
                v = work.tile([128, 5, C], F32R, name=f"v{b}", tag="v", bufs=1)
                for mi, (m0, mw) in enumerate(MC):
                    ps = psum.tile([128, 2, 512], F32, name=f"psv{b}_{mi}", tag="mm", bufs=2)
                    for k in range(KT):
                        for h, dst in enumerate(two(ps[:mw], CH)):
                            nc.tensor.matmul(
                                dst,
                                xT[:, k, m0 : m0 + mw],
                                wq_sb[:, k, 2 * C + h * CH : 2 * C + (h + 1) * CH],
                                start=(k == 0),
                                stop=(k == KT - 1),
                            )
                    nc.vector.tensor_copy(
                        v[:mw, mi, :].rearrange("p (h w) -> p h w", h=2),
                        ps[:mw, :, :CH],
                    )

                # ---- sT[m, n] = k @ q.T ; ET = exp(sT * scale) ----
                et = work.tile([128, 5, NP], F32R, name=f"et{b}", tag="et", bufs=1)
                for mi, (m0, mw) in enumerate(MC):
                    ps = psum.tile([128, 2, 512], F32, name=f"pss{b}_{mi}", tag="mm", bufs=2)
                    for k in range(KT):
                        for h, dst in enumerate(two(ps[:mw], NH)):
                            nc.tensor.matmul(
                                dst,
                                qk[:, 6 + k, m0 : m0 + mw],
                                qk[:, k, h * NH : (h + 1) * NH],
                                start=(k == 0),
                                stop=(k == KT - 1),
                            )
                    nc.scalar.activation(
                        et[:mw, mi, :].rearrange("p (h w) -> p h w", h=2),
                        ps[:mw, :, :NH],
                        mybir.ActivationFunctionType.Exp,
                        scale=SCALE,
                    )

                # ---- r[n] = sum_m ET ; rinv ; broadcast to all partitions ----
                ps_r = psum.tile([1, 2, 512], F32, name=f"psr{b}", tag="mm", bufs=2)
                for mi, (m0, mw) in enumerate(MC):
                    for h, dst in enumerate(two(ps_r, NH)):
                        nc.tensor.matmul(
                            dst,
                            ones_sb[:mw, 0:1],
                            et[:mw, mi, h * NH : (h + 1) * NH],
                            start=(mi == 0),
                            stop=(mi == 4),
                        )
                rinv = work.tile([1, NP], F32, name=f"rinv{b}", tag="rinv", bufs=1)
                nc.vector.reciprocal(
                    rinv.rearrange("p (h w) -> p h w", h=2), ps_r[:, :, :NH]
                )
                rinv_r = work.tile([1, NP], F32R, name=f"rinvr{b}", tag="rinvr", bufs=1)
                nc.vector.tensor_copy(rinv_r, rinv)
                ps_rb = psum.tile([128, 2, 512], F32, name=f"psrb{b}", tag="rb", bufs=1)
                for h, dst in enumerate(two(ps_rb, NH)):
                    nc.tensor.matmul(
                        dst,
                        ones_sb[0:1, :],
                        rinv_r[0:1, h * NH : (h + 1) * NH],
                        start=True,
                        stop=True,
                    )
                rb = work.tile([128, NP], F32, name=f"rb{b}", tag="rb_sb", bufs=1)
                nc.vector.tensor_copy(
                    rb.rearrange("p (h w) -> p h w", h=2), ps_rb[:, :, :NH]
                )

                # ---- token_attn = attn[0, 1:] = ET[1:, 0] * rinv[0] ----
                ta = work.tile([128, 5], F32, name=f"ta{b}", tag="ta", bufs=1)
                for mi, (m0, mw) in enumerate(MC):
                    nc.vector.tensor_mul(
                        ta[:mw, mi : mi + 1],
                        et.bitcast(F32)[:mw, mi, 0:1],
                        rb[:mw, 0:1],
                    )
                for mi, (m0, mw) in enumerate(MC):
                    lo = 1 if mi == 0 else 0
                    nc.sync.dma_start(
                        out=ta_d.ap()[b, m0 + lo - 1 : m0 + mw - 1],
                        in_=ta[lo:mw, mi : mi + 1],
                    )

                # ---- yT[c, n] = v.T @ ET, normalized by rinv[n] at evac ----
                z_t = dram.tile([5 * 128 * C], F32, name=f"z{b}", tag="z", bufs=2)
                z_store = z_t[: C * N].rearrange("(c p n) -> p c n", p=128, n=N)
                z_load = z_t.rearrange("(m p c) -> p m c", p=128, c=C)
                for c in range(KT):
                    ps = psum.tile([128, 2, 512], F32, name=f"psy{b}_{c}", tag="mm", bufs=2)
                    for mi, (m0, mw) in enumerate(MC):
                        for h, dst in enumerate(two(ps, NH)):
                            nc.tensor.matmul(
                                dst,
                                v[:mw, mi, c * 128 : (c + 1) * 128],
                                et[:mw, mi, h * NH : (h + 1) * NH],
                                start=(mi == 0),
                                stop=(mi == 4),
                            )
                    yt = work.tile([128, NP], F32, name=f"yt{b}_{c}", tag="yt", bufs=2)
                    nc.vector.tensor_mul(
                        yt.rearrange("p (h w) -> p h w", h=2),
                        ps[:, :, :NH],
                        rb.rearrange("p (h w) -> p h w", h=2),
                    )
                    nc.sync.dma_start(out=z_store[:, c, :], in_=yt[:, :N])

                # ---- zT[c', n'] via PE transpose of z tiles ----
                zT = work.tile([128, KT, N], F32R, name=f"zT{b}", tag="zT", bufs=1)
                for mi, (m0, mw) in enumerate(MC):
                    zm = work.tile([128, C], F32, name=f"zm{b}_{mi}", tag="zm", bufs=2)
                    nc.sync.dma_start(out=zm[:mw, :], in_=z_load[:mw, mi, :])
                    for c in range(KT):
                        pst = psum.tile([128, 128], F32, name=f"pst{b}_{mi}_{c}", tag="tp", bufs=2)
                        nc.tensor.transpose(
                            pst[:, :mw],
                            zm[:mw, c * 128 : (c + 1) * 128],
                            ident[:mw, :mw],
                        )
                        nc.scalar.activation(
                            zT[:, c, m0 : m0 + mw], pst[:, :mw],
                            mybir.ActivationFunctionType.Copy,
                        )

                # ---- out[n, c] = zT.T @ w_projT + b_proj ----
                for mi, (m0, mw) in enumerate(MC):
                    ps = psum.tile([128, 2, 512], F32, name=f"pso{b}_{mi}", tag="mm", bufs=2)
                    for h, dst in enumerate(two(ps[:mw], CH)):
                        nc.tensor.matmul(
                            dst,
                            ones_sb[0:1, :mw],
                            bp_sb[0:1, h * CH : (h + 1) * CH],
                            start=True,
                            stop=False,
                        )
                        for c in range(KT):
                            nc.tensor.matmul(
                                dst,
                                zT[:, c, m0 : m0 + mw],
                                wp_sb[:, c, h * CH : (h + 1) * CH],
                                start=False,
                                stop=(c == KT - 1),
                            )
                    ob = work.tile([128, C], F32, name=f"ob{b}_{mi}", tag="ob", bufs=2)
                    nc.vector.tensor_copy(
                        ob[:mw, :].rearrange("p (h w) -> p h w", h=2),
                        ps[:mw, :, :CH],
                    )
                    nc.sync.dma_start(out=out_d.ap()[b, m0 : m0 + mw, :], in_=ob[:mw, :])

    nc.finalize()
    return nc


_NC_CACHE = None


def _get_nc():
    global _NC_CACHE
    if _NC_CACHE is None:
        _NC_CACHE = build_kernel()
    return _NC_CACHE


def kernel(x, w_qkv, w_proj, b_proj):
    x = np.asarray(x, dtype=np.float32)
    w_qkv = np.asarray(w_qkv, dtype=np.float32)
    w_proj = np.asarray(w_proj, dtype=np.float32)
    b_proj = np.asarray(b_proj, dtype=np.float32)

    # host-side layout prep (cheap relative to device exec)
    xT = np.zeros((B, C, NP), dtype=np.float32)
    xT[:, :, :N] = np.transpose(x, (0, 2, 1))
    w_qkvT = np.ascontiguousarray(w_qkv.T)  # [C, 3C]
    w_projT = np.ascontiguousarray(w_proj.T)  # [C, C]
    bp = np.ascontiguousarray(b_proj.reshape(1, C))
    ones = np.ones((128, 128), dtype=np.float32)

    nc = _get_nc()
    in_maps = [
        {
            "xT": np.ascontiguousarray(xT[i * BPC : (i + 1) * BPC]),
            "w_qkvT": w_qkvT,
            "w_projT": w_projT,
            "b_proj": bp,
            "ones": ones,
        }
        for i in range(NCORES)
    ]
    res = run_bass_kernel_spmd(nc, in_maps, core_ids=list(range(NCORES)), trace=False)

    out = np.concatenate([r["out"] for r in res.results], axis=0)
    ta = np.concatenate([r["token_attn"] for r in res.results], axis=0)
    attn_size = int((N - 1) ** 0.5)
    token_attn = ta.reshape(B, 1, attn_size, attn_size)
    return out, token_attn
